# revision 2
# baseline (speedup 1.0000x reference)
"""Trainium2 Bass kernel for nn_DictNet loss (8-core SPMD), v6.

Math restructuring (same as v1)
-------------------------------
  Cn    = C / ||C||                      (tiny, host)
  L     = einsum('nmk,k->nm', D, Cn)     (memory-bound: 738 MB of D)
  y_hat = x - L @ x
  d     = pairwise distance matrix of y_hat rows   [N, N]
  loss  = sparsity(Cn) + sum_c u_c d u_c^T - (1/(S^2*beta)) * sum_g h_g d h_g^T

v6 = v2 + bf16-cast D stream
----------------------------
* D tiles are cast fp32 -> bf16 during the DMA (SWDGE cast path): HBM reads
  are unchanged but the DVE AXPY (the hardware phase-A bottleneck) runs at
  the 2x 16-bit rate; the L^T fold is a regular matmul against a bf16
  identity so the PSUM stays fp32 (plain TRN2 ISA).

v2 performance restructure
--------------------------
* y_hat is accumulated TRANSPOSED (y^T[f, own-rows]) directly in PSUM by
  swapping the matmul operands (lhsT = x block, rhs = L^T block), which
  deletes the whole post-phase-A transpose stage.
* x is loaded once (bf16, SBUF-resident) instead of re-streamed per m-group.
* Everything downstream of y_hat is bf16: the AllGather payload, the y^T
  tiles, the gram/vu/vh matmul operands and the distance tiles. PSUM math
  stays fp32.
* All small phase-D weights are DMA'd during the D stream (front-loaded).
* The last m-group is split into two 256-wide groups so the post-DMA AXPY
  drain is short.

Sharding: D rows (node axis) split across 8 cores; y_hat^T AllGathered so
every core forms distance tiles for its own rows. Symmetry: each core only
processes JBLK = CORES/2 + 1 rotated column blocks; off-diagonal blocks are
double-counted via host-scaled weights; the j = CORES/2 block is
zero-weighted on the upper half of the cores.
"""

import math

import numpy as np

import concourse.bass as bass
import concourse.mybir as mybir
import concourse.tile as tile
from concourse import bacc
from concourse.bass_utils import run_bass_kernel_spmd
from concourse.masks import make_identity

FP32 = mybir.dt.float32
BF16 = mybir.dt.bfloat16
AF = mybir.ActivationFunctionType
OP = mybir.AluOpType

FULL_CFG = dict(N=4096, F=512, K=11, G=128, NCLS=7, CORES=8)


def _derived(cfg):
    N, F, K, G, NCLS, CORES = (
        cfg["N"], cfg["F"], cfg["K"], cfg["G"], cfg["NCLS"], cfg["CORES"])
    R = N // CORES              # rows per core
    assert R % 128 == 0 and N % 512 == 0 and F % 128 == 0
    NRC = R // 128              # 128-row chunks per core
    NMC = N // 128              # 128-col chunks (m axis)
    # m-axis D tile groups: 512-wide, last one split into 2x256 to shorten
    # the post-DMA drain
    MR = [(m, 512) for m in range(0, N - 512, 512)]
    MR += [(N - 512, 256), (N - 256, 256)]
    NFC = F // 128              # feature chunks
    XSUB = N // 128             # m sub-blocks in resident x
    JBLK = CORES // 2 + 1       # rotated col blocks each core processes
    return dict(N=N, F=F, K=K, G=G, NCLS=NCLS, CORES=CORES, R=R, NRC=NRC,
                NMC=NMC, MR=MR, NFC=NFC, XSUB=XSUB, JBLK=JBLK)


def build(cfg, reps=1, stage="full", chained=False):
    """Build the SPMD kernel (one NEFF, runs on all cores).

    reps > 1 repeats the whole computation serially (timing probe).
    stage: "dma" = D loads only, "axpy" = + AXPY, "A" = phases A+B,
    "AG"/"simAG" = + collective (simAG fakes it), "sim" = full with faked
    collective (for TimelineSim), "full" = everything.
    """
    c = _derived(cfg)
    N, F, K, G, NCLS = c["N"], c["F"], c["K"], c["G"], c["NCLS"]
    CORES, R, NRC, NMC = c["CORES"], c["R"], c["NRC"], c["NMC"]
    MR, NFC, XSUB, JBLK = c["MR"], c["NFC"], c["XSUB"], c["JBLK"]

    nc = bacc.Bacc("TRN2", target_bir_lowering=False, debug=False,
                   num_devices=CORES)

    # ---- I/O ----
    Dsh = nc.dram_tensor("Dsh", [R, N, K], FP32, kind="ExternalInput")
    x_in = nc.dram_tensor("x_in", [N, F], BF16, kind="ExternalInput")
    xT_in = nc.dram_tensor("xT_own", [F, R], BF16, kind="ExternalInput")
    cnb_in = nc.dram_tensor("cnb", [128, K], FP32, kind="ExternalInput")
    uT_in = nc.dram_tensor("uT_sh", [R, NCLS], BF16, kind="ExternalInput")
    hT_in = nc.dram_tensor("hT_sh", [R, G], BF16, kind="ExternalInput")
    u_in = nc.dram_tensor("u_rot", [NCLS, JBLK, R], FP32, kind="ExternalInput")
    h_in = nc.dram_tensor("h_rot", [G, JBLK, R], FP32, kind="ExternalInput")
    dmask_in = nc.dram_tensor("dmask", [128, NRC, R], BF16, kind="ExternalInput")
    out_u = nc.dram_tensor("out_u", [NCLS, JBLK], FP32, kind="ExternalOutput")
    out_h = nc.dram_tensor("out_h", [G, JBLK], FP32, kind="ExternalOutput")

    # cross-rep serialization bounce for single-shot timing (chained=True)
    chain = nc.dram_tensor("chain", [1, 1], FP32)
    # collective bounce buffers: rows 0..F-1 = y_hat^T (own cols), row F = sn
    agin = nc.dram_tensor("agin", [F + 1, R], BF16)
    agout = nc.dram_tensor("agout", [CORES, F + 1, R], BF16,
                           addr_space="Shared")

    with tile.TileContext(nc) as tc:
      for rep in range(reps):
          with tc.tile_pool(name=f"persist{rep}", bufs=1) as pp:
              identf = pp.tile([128, 128], FP32)
              make_identity(nc, identf[:])
              ident = pp.tile([128, 128], BF16)
              nc.vector.tensor_copy(ident[:], identf[:])
              cnb = pp.tile([128, K], FP32)
              nc.sync.dma_start(cnb[:], cnb_in[:])
              if chained and rep > 0:
                  # rep k's first consumer waits on rep k-1's last result:
                  # cnb[0,0] = 0*chain + cnb[0,0] forces the dependency
                  # through real dataflow without changing the value
                  cht = pp.tile([1, 1], FP32, name=f"cht{rep}")
                  nc.sync.dma_start(cht[:], chain[:])
                  nc.vector.scalar_tensor_tensor(
                      cnb[0:1, 0:1], cht[:], 0.0, cnb[0:1, 0:1],
                      OP.mult, OP.add)

              # constants: ones in bf16 (memset fp32 then cast-copy)
              ones_f = pp.tile([1, 128], FP32)
              nc.vector.memset(ones_f[:], 1.0)
              ones_row = pp.tile([1, 128], BF16)   # [1,128] lhsT broadcaster
              nc.vector.tensor_copy(ones_row[:], ones_f[:])
              onesc_f = pp.tile([128, 1], FP32)
              nc.vector.memset(onesc_f[:], 1.0)
              ones_col = pp.tile([128, 1], BF16)   # [128,1] column reducer
              nc.vector.tensor_copy(ones_col[:], onesc_f[:])

              yT_own = [pp.tile([128, R], BF16, tag=f"yT{fc}",
                                name=f"yT_own{rep}_{fc}")
                        for fc in range(NFC)]
              sn_own = [pp.tile([128, 1], FP32, tag=f"sn{rc}",
                                name=f"sn_own{rep}_{rc}")
                        for rc in range(NRC)]
              sn_sb = pp.tile([1, R], BF16, name=f"sn_sb{rep}")
              acc_u = pp.tile([NCLS, JBLK], FP32)
              acc_h = pp.tile([G, JBLK], FP32)
              if stage not in ("full", "sim"):
                  nc.vector.memset(acc_u[:], 0.0)
                  nc.vector.memset(acc_h[:], 0.0)

              # ------------- Phase A: L = sum_k cn_k * D_k; yT -= (Lx)^T ----
              with (
                  tc.tile_pool(name=f"psYT{rep}", bufs=1, space="PSUM") as psYT,
              ):
                  ytpsum = [psYT.tile([128, R], FP32, tag=f"ytp{fc}",
                                      name=f"ytpsum{rep}_{fc}")
                            for fc in range(NFC)]
                  with (
                      tc.tile_pool(name=f"dA{rep}", bufs=6) as dpool,
                      tc.tile_pool(name=f"lA{rep}", bufs=2 * NRC + 1) as lpool,
                      tc.tile_pool(name=f"ltA{rep}", bufs=3) as ltsb_pool,
                      tc.tile_pool(name=f"psLT{rep}", bufs=4,
                                   space="PSUM") as psLT,
                  ):
                      # software-pipelined D-tile DMA issue: the queue is
                      # FIFO, so the big x load and the small phase-B/D
                      # weights slot in behind the first D group instead of
                      # delaying it
                      flat = [(gi, rc) for gi in range(len(MR))
                              for rc in range(NRC)]
                      dtile = {}
                      issued = [0]

                      def issue_d(n):
                          for _ in range(n):
                              if issued[0] >= len(flat):
                                  return
                              gi, rc = flat[issued[0]]
                              m0, mw = MR[gi]
                              t = dpool.tile([128, 512, K], BF16, tag="D")
                              nc.gpsimd.dma_start(
                                  t[:, :mw, :],
                                  Dsh[rc * 128:(rc + 1) * 128, m0:m0 + mw, :])
                              dtile[(gi, rc)] = t
                              issued[0] += 1

                      issue_d(NRC)  # group 0 ahead of everything else
                      # resident x (bf16): [p, m-sub, f]
                      x_sb = pp.tile([128, XSUB, F], BF16, name=f"x_sb{rep}")
                      nc.sync.dma_start(
                          x_sb[:], x_in[:].rearrange("(s p) f -> p s f", p=128))
                      issue_d(2)
                      # small phase-B/D operands, loaded under the D stream
                      xT_sb = pp.tile([128, NFC, R], BF16, name=f"xT_sb{rep}")
                      nc.sync.dma_start(
                          xT_sb[:],
                          xT_in[:].rearrange("(fc p) n -> p fc n", p=128))
                      uT_sb = pp.tile([128, NRC, NCLS], BF16,
                                      name=f"uT_sb{rep}")
                      nc.sync.dma_start(
                          uT_sb[:],
                          uT_in[:].rearrange("(rc p) c -> p rc c", p=128))
                      hT_sb = pp.tile([128, NRC, G], BF16, name=f"hT_sb{rep}")
                      nc.sync.dma_start(
                          hT_sb[:],
                          hT_in[:].rearrange("(rc p) g -> p rc g", p=128))
                      u_sb = pp.tile([NCLS, JBLK, R], FP32, name=f"u_sb{rep}")
                      nc.sync.dma_start(u_sb[:], u_in[:])
                      h_sb = pp.tile([G, JBLK, R], FP32, name=f"h_sb{rep}")
                      nc.sync.dma_start(h_sb[:], h_in[:])
                      dmask = pp.tile([128, NRC, R], BF16, name=f"dmask{rep}")
                      nc.sync.dma_start(dmask[:], dmask_in[:])

                      mc = 0
                      for gi, (m0, mw) in enumerate(MR):
                          subs = mw // 128
                          lgs = []
                          for rc in range(NRC):
                              issue_d(1)
                              if stage == "dma":
                                  dtile.pop((gi, rc), None)
                                  continue
                              dt = dtile.pop((gi, rc))
                              lg = lpool.tile([128, 512], BF16, tag="L",
                                              name=f"lg{rep}_{gi}_{rc}")
                              nc.vector.tensor_scalar_mul(
                                  lg[:, :mw], dt[:, :mw, 0], cnb[:, 0:1])
                              for k in range(1, K):
                                  nc.vector.scalar_tensor_tensor(
                                      lg[:, :mw], dt[:, :mw, k],
                                      cnb[:, k:k + 1], lg[:, :mw],
                                      OP.mult, OP.add)
                              lgs.append(lg)
                          if stage in ("dma", "axpy"):
                              mc += subs
                              continue
                          for sub in range(subs):
                              ltp = psLT.tile([128, R], FP32, tag="LT",
                                              name=f"ltp{rep}_{gi}_{sub}")
                              for rc in range(NRC):
                                  nc.tensor.matmul(
                                      ltp[:, rc * 128:(rc + 1) * 128],
                                      lhsT=lgs[rc][:, sub * 128:(sub + 1) * 128],
                                      rhs=ident[:], start=True, stop=True)
                              lts = ltsb_pool.tile([128, R], BF16, tag="LTS")
                              nc.scalar.copy(lts[:], ltp[:])
                              for fc in range(NFC):
                                  nc.tensor.matmul(
                                      ytpsum[fc][:],
                                      lhsT=x_sb[:, mc, fc * 128:(fc + 1) * 128],
                                      rhs=lts[:],
                                      start=(mc == 0), stop=(mc == NMC - 1))
                              mc += 1

                  if stage in ("dma", "axpy"):
                      nc.vector.memset(acc_u[:], 0.0)
                      nc.vector.memset(acc_h[:], 0.0)
                      nc.sync.dma_start(out_u[:], acc_u[:])
                      nc.sync.dma_start(out_h[:], acc_h[:])
                      continue

                  # ---- Phase B: y^T = x^T - (Lx)^T; sn; stage AllGather ----
                  with (
                      tc.tile_pool(name=f"sqB{rep}", bufs=2) as sqB,
                      tc.tile_pool(name=f"psB{rep}", bufs=2,
                                   space="PSUM") as psB,
                  ):
                      snp = psB.tile([1, R], FP32, name=f"snp{rep}")
                      for fc in range(NFC):
                          nc.vector.scalar_tensor_tensor(
                              yT_own[fc][:], ytpsum[fc][:], -1.0,
                              xT_sb[:, fc, :], OP.mult, OP.add)
                          nc.sync.dma_start(
                              agin[fc * 128:(fc + 1) * 128, :], yT_own[fc][:])
                          sq = sqB.tile([128, R], BF16, tag="sq")
                          nc.scalar.activation(sq[:], yT_own[fc][:], AF.Square)
                          nc.tensor.matmul(
                              snp[:], lhsT=ones_col[:], rhs=sq[:],
                              start=(fc == 0), stop=(fc == NFC - 1))
                      nc.scalar.copy(sn_sb[:], snp[:])
                      nc.sync.dma_start(agin[F:F + 1, :], sn_sb[:])
                      # sn columns [128,1] per own rc chunk (1-contraction MM)
                      onesp = sqB.tile([1, 1], BF16, tag="o1")
                      nc.vector.tensor_copy(onesp[:], ones_f[:, 0:1])
                      for rc in range(NRC):
                          snc = psB.tile([128, 1], FP32, tag="snc")
                          nc.tensor.matmul(
                              snc[:],
                              lhsT=sn_sb[0:1, rc * 128:(rc + 1) * 128],
                              rhs=onesp[:], start=True, stop=True)
                          nc.scalar.copy(sn_own[rc][:], snc[:])

              if stage == "A":
                  nc.sync.dma_start(out_u[:], acc_u[:])
                  nc.sync.dma_start(out_h[:], acc_h[:])
                  continue

              # ---------------- AllGather y_hat^T + sn ----------------
              if stage in ("sim", "simAG"):
                  # TimelineSim can't run collectives: stand in DMAs with
                  # equivalent traffic.
                  for r in range(CORES):
                      nc.sync.dma_start(agout[r], agin[:])
              else:
                  nc.gpsimd.collective_compute(
                      "AllGather", OP.bypass,
                      replica_groups=[list(range(CORES))],
                      ins=[agin[:]], outs=[agout[0:CORES]])

              if stage in ("AG", "simAG"):
                  nc.sync.dma_start(out_u[:], acc_u[:])
                  nc.sync.dma_start(out_h[:], acc_h[:])
                  continue

              # ---------------- Phase D: distance tiles + weighted sums -----
              sp_eng = nc.engines[mybir.EngineType.SP]
              pid = sp_eng.partition_id()
              rot = []  # SP registers holding (pid + j) % CORES for j >= 1
              for j in range(1, JBLK):
                  rj = sp_eng.alloc_register(f"rot{rep}_{j}")
                  sp_eng.reg_alu(rj, pid, j, OP.add)
                  sp_eng.reg_alu(rj, rj, CORES, OP.mod)
                  rot.append(bass.make_scalar_value(rj, min_val=0,
                                                    max_val=CORES - 1))
              with (
                  tc.tile_pool(name=f"yTD{rep}", bufs=1) as ytd_pool,
                  tc.tile_pool(name=f"snD{rep}", bufs=1) as sn_pool,
                  tc.tile_pool(name=f"sqD{rep}", bufs=4) as sqd_pool,
                  tc.tile_pool(name=f"dD{rep}", bufs=4) as dd_pool,
                  tc.tile_pool(name=f"ttD{rep}", bufs=2) as tt_pool,
                  tc.tile_pool(name=f"psG{rep}", bufs=3, space="PSUM") as psG,
                  tc.tile_pool(name=f"psV{rep}", bufs=2, space="PSUM") as psV,
                  tc.tile_pool(name=f"psS{rep}", bufs=1, space="PSUM") as psS,
              ):
                  # j-major loads so the j=1 block lands first
                  yT_rot = [ytd_pool.tile([128, JBLK - 1, R], BF16,
                                          tag=f"yTr{fc}",
                                          name=f"yT_rot{rep}_{fc}")
                            for fc in range(NFC)]
                  sn_rot = sn_pool.tile([1, JBLK - 1, R], BF16)
                  for j in range(1, JBLK):
                      for fc in range(NFC):
                          nc.sync.dma_start(
                              yT_rot[fc][:, j - 1, :],
                              agout[bass.ds(rot[j - 1], 1),
                                    fc * 128:(fc + 1) * 128, :]
                              .rearrange("r f n -> f (r n)"))
                      nc.sync.dma_start(
                          sn_rot[:, j - 1, :],
                          agout[bass.ds(rot[j - 1], 1), F:F + 1, :]
                          .rearrange("r one n -> one (r n)"))
                  # broadcast sn rows to [128, R] per j block
                  sncol = sn_pool.tile([128, JBLK, R], FP32)
                  for j in range(JBLK):
                      snb = psS.tile([128, R], FP32, tag="snb")
                      src_row = (sn_sb[:] if j == 0
                                 else sn_rot[:, j - 1, :])
                      nc.tensor.matmul(snb[:], lhsT=ones_row[:], rhs=src_row,
                                       start=True, stop=True)
                      nc.scalar.copy(sncol[:, j, :], snb[:])

                  tiles = [(j, rc) for j in range(JBLK) for rc in range(NRC)]
                  vu = vh = None
                  pending = None  # (j, rc, d_tile) awaiting V matmuls

                  def flush_pending():
                      nonlocal pending
                      if pending is None:
                          return
                      pj, prc, pdt = pending
                      nc.tensor.matmul(
                          vu[:], lhsT=uT_sb[:, prc, :], rhs=pdt[:],
                          start=(prc == 0), stop=(prc == NRC - 1))
                      nc.tensor.matmul(
                          vh[:], lhsT=hT_sb[:, prc, :], rhs=pdt[:],
                          start=(prc == 0), stop=(prc == NRC - 1))
                      pending = None
                      if prc == NRC - 1:
                          su = tt_pool.tile([NCLS, R], FP32, tag="su",
                                            name=f"su{rep}_{pj}")
                          nc.vector.tensor_tensor(
                              out=su[:], in0=vu[:], in1=u_sb[:, pj, :],
                              op=OP.mult)
                          nc.vector.reduce_sum(
                              acc_u[:, pj:pj + 1], su[:],
                              axis=mybir.AxisListType.X)
                          sh = tt_pool.tile([G, R], FP32, tag="sh",
                                            name=f"sh{rep}_{pj}")
                          nc.vector.tensor_tensor(
                              out=sh[:], in0=vh[:], in1=h_sb[:, pj, :],
                              op=OP.mult)
                          nc.vector.reduce_sum(
                              acc_h[:, pj:pj + 1], sh[:],
                              axis=mybir.AxisListType.X)

                  for j, rc in tiles:
                      if rc == 0:
                          new_vu = psV.tile([NCLS, R], FP32, tag="vu",
                                            name=f"vu{rep}_{j}")
                          new_vh = psV.tile([G, R], FP32, tag="vh",
                                            name=f"vh{rep}_{j}")
                      gram = psG.tile([128, R], FP32, tag="g",
                                      name=f"gram{rep}_{j}_{rc}")
                      for fc in range(NFC):
                          rhs = (yT_own[fc][:] if j == 0
                                 else yT_rot[fc][:, j - 1, :])
                          nc.tensor.matmul(
                              gram[:],
                              lhsT=yT_own[fc][:, rc * 128:(rc + 1) * 128],
                              rhs=rhs,
                              start=(fc == 0), stop=(fc == NFC - 1))
                      flush_pending()
                      if rc == 0:
                          vu, vh = new_vu, new_vh
                      sq = sqd_pool.tile([128, R], FP32, tag="sq")
                      nc.vector.scalar_tensor_tensor(
                          sq[:], gram[:], -2.0, sncol[:, j, :],
                          OP.mult, OP.add)
                      nc.vector.tensor_scalar(
                          sq[:], sq[:], sn_own[rc][:], 0.0, OP.add, OP.max)
                      dt = dd_pool.tile([128, R], BF16, tag="d")
                      nc.scalar.activation(dt[:], sq[:], AF.Sqrt)
                      if j == 0:
                          nc.vector.tensor_tensor(
                              out=dt[:], in0=dt[:], in1=dmask[:, rc, :],
                              op=OP.mult)
                      pending = (j, rc, dt)
                  flush_pending()

                  nc.sync.dma_start(out_u[:], acc_u[:])
                  nc.sync.dma_start(out_h[:], acc_h[:])
                  if chained:
                      nc.sync.dma_start(chain[:], acc_u[0:1, 0:1])

    nc.compile()
    return nc


def host_prep(cfg, D, x, C, mask, y, groups):
    """Host-side input prep: normalize C, build weight matrices, shard."""
    c = _derived(cfg)
    N, K, G, NCLS, CORES, R = c["N"], c["K"], c["G"], c["NCLS"], c["CORES"], c["R"]
    NRC, JBLK = c["NRC"], c["JBLK"]
    bf16 = mybir.dt.np(BF16)

    C32 = np.asarray(C, np.float32)
    cn = (C32 / np.linalg.norm(C32, axis=0, keepdims=True)).astype(np.float32)
    dim = np.float32(math.sqrt(K))
    nrm = np.linalg.norm(cn, axis=0).astype(np.float32)
    sparsity = float(np.mean((dim - np.abs(cn).sum(0) / nrm) / (dim - 1.0)))

    mask_b = np.asarray(mask, bool)
    y_i = np.asarray(y, np.int64)
    cnt = np.zeros(NCLS, np.int64)
    np.add.at(cnt, y_i[mask_b], 1)
    u = np.zeros((NCLS, N), np.float32)
    sel = mask_b & (cnt[y_i] > 0)
    u[y_i[sel], np.nonzero(sel)[0]] = 1.0 / cnt[y_i[sel]]

    g_i = np.asarray(groups, np.int64)
    H = np.zeros((G, N), np.float32)
    np.add.at(H, (np.repeat(np.arange(G), g_i.shape[1]), g_i.ravel()), 1.0)

    cnb = np.tile(cn.ravel()[None, :], (128, 1)).astype(np.float32)
    uT = np.ascontiguousarray(u.T).astype(bf16)
    hT = np.ascontiguousarray(H.T).astype(bf16)
    x32 = np.ascontiguousarray(np.asarray(x, np.float32))
    x16 = x32.astype(bf16)
    D32 = np.asarray(D, np.float32)

    # diagonal mask for the j=0 (own) block: 0 at global col == global row
    dmask = np.ones((128, NRC, R), np.float32)
    for rc in range(NRC):
        for p in range(128):
            dmask[p, rc, rc * 128 + p] = 0.0
    dmask = dmask.astype(bf16)

    in_maps = []
    for ci in range(CORES):
        sl = slice(ci * R, (ci + 1) * R)
        # rotated, symmetry-scaled weight slices: j -> global block (ci+j)%CORES
        u_rot = np.zeros((NCLS, JBLK, R), np.float32)
        h_rot = np.zeros((G, JBLK, R), np.float32)
        for j in range(JBLK):
            gb = (ci + j) % CORES
            scale = 1.0 if j == 0 else 2.0
            if j == CORES // 2 and ci >= CORES // 2:
                continue  # pair already handled by core ci - CORES//2
            u_rot[:, j, :] = u[:, gb * R:(gb + 1) * R] * scale
            h_rot[:, j, :] = H[:, gb * R:(gb + 1) * R] * scale
        in_maps.append({
            "Dsh": np.ascontiguousarray(D32[sl]),
            "x_in": x16,
            "xT_own": np.ascontiguousarray(x32[sl].T).astype(bf16),
            "cnb": cnb,
            "uT_sh": np.ascontiguousarray(uT[sl]),
            "hT_sh": np.ascontiguousarray(hT[sl]),
            "u_rot": u_rot,
            "h_rot": h_rot,
            "dmask": dmask,
        })
    return in_maps, sparsity


def combine(cfg, results, sparsity, group_size):
    """loss = sparsity + hl2 + hl1/beta, from per-core partial sums."""
    beta = np.float64(cfg["G"]) / np.float64(cfg["NCLS"])
    hl2 = np.float64(0.0)
    s1 = np.float64(0.0)
    for r in results:
        hl2 += r["out_u"].astype(np.float64).sum()
        s1 += r["out_h"].astype(np.float64).sum()
    hl1 = -s1 / np.float64(group_size * group_size)
    total = np.float64(sparsity) + hl2 + hl1 / beta
    return np.float32(total)


_BUILD_CACHE = {}


def _get_nc(key, cfg):
    if key not in _BUILD_CACHE:
        _BUILD_CACHE[key] = build(cfg)
    return _BUILD_CACHE[key]


def kernel(D, x, C, mask, y, groups):
    cfg = dict(FULL_CFG)
    in_maps, sparsity = host_prep(cfg, D, x, C, mask, y, groups)
    nc = _get_nc("full", cfg)
    res = run_bass_kernel_spmd(
        nc, in_maps, core_ids=list(range(cfg["CORES"])), trace=False)
    return combine(cfg, res.results, sparsity, np.asarray(groups).shape[1])
